# revision 65
# baseline (speedup 1.0000x reference)
"""Trainium2 Bass kernel for nms_detection (GaussianBlur5x5 -> MaxPool3x3 -> peak NMS + threshold).

Contract: kernel(hands_batch) takes the FULL [256, 2, 224, 398] f32 input and
returns the FULL [256, 2, 224, 398] f32 peaks map. Internally data-parallel
over 8 NeuronCores: 512 planes -> 64 planes/core.

v2 of the previous 340us kernel: the entire horizontal-max + compare + select
tail (t2/m2/is_ge/mult = ~2.4 DVE passes + 0.6 Pool pass + edge stts in v1)
is replaced by a 3-instruction chain of ONE custom DVE op
  GATE(a, b) = b * (b >= a)
using the zero-annihilation trick: since vm >= THRP > 0 everywhere, a zeroed
value can never pass a later gate with a positive product, so
  e1 = GATE(vm,    bd)   (center-column test; bd = blur center row)
  e2 = GATE(vm>>1, e1)   (right neighbour; e1==0 rows stay 0)
  out = GATE(vm<<1, e2)  (left neighbour; emitted directly as bf16)
equals bd * [bd >= max3x3(blur), bd >= THRP] exactly (fp32 compares; the
only rounding is the final bf16 value store, same as v1). The vm tile keeps
-1e30 pad columns at both ends so no edge-column instructions are needed.

Per-core algorithm (plane = one [224, 398] image channel):
  - Rows live on SBUF partitions; H=224 splits into two chunks of 113 blur
    rows (+1 duplicated edge row -> M=114); 4 planes per supertile.
  - Blur: 3 accumulating fp32 matmuls per plane-chunk on the PE via gaussian
    symmetry gh=[a,b,c,b,a]: blur = (c*Gv)@x0 + (b*Gv)@s1 + (a*Gv)@s2 with
    s1=x[-1]+x[+1], s2=x[-2]+x[+2]. fp32 is mandatory: f32r/bf16/f16 matmuls
    measure ~11-bit mantissa on HW, flipping ~10k near-tie NMS compares (the
    2e-2 rel-err budget only allows ~500).
  - s1/s2 pre-adds on the Pool engine (gpsimd; HW ISA = add/mult/memset only,
    max/is_ge/stt/copy are rejected), ACT does PSUM->SBUF blur copies +
    reflect edge-column scale-copies.
  - vertical 3x1 max on DVE: two DMA-materialized partition-shifted copies
    (engines cannot read operands at mismatched partition offsets), then
    t1 = max(blur, blurdn, THRP) (stt) and vm = max(t1, blur2).
  - horizontal max + NMS compare + select: the 3 GATE instructions above.
Schedule: 3-stage software pipeline (emission order = tile-scheduler
priority): consume_mm(k) emitted BEFORE produce(k+LEAD) each step so the
shift-copy DMAs of iter k enqueue ahead of the input load of iter k+LEAD on
the serialized DMA device (the chain's critical dependency); produce leads
consume_mm by LEAD=2 (xin/ssum bufs=LEAD+2 breaks the Pool<->PE circular
wait through the s1/s2 buffer rotation that periodically stalled PE and
reset its p-state ramp; xin at LEAD+3=5 bufs gives the input loads extra
slack on the serialized DMA device, -701ns), chain trails by PRE=1. Shift copies issue from the
ACT queue (HWDGE), loads/stores from SP. DMA batched via transposed DRAM
access patterns. The first two iterations' s1/s2 run on the DVE (idle during
pipeline fill, and 2x Pool's rate), pulling the j=1/j=0 matmuls earlier;
the last iteration's chain+store is 2-plane-sliced so the final store
overlaps the final gates (drain); ALL stores issue as 2-plane slices for
finer interleave on the serialized DMA device (-200ns; 1-plane is worse).
TimelineSim: 306822ns e2e (DVE 275us busy at 97.5% occupancy = the wall:
t1/vm/3 gates x 32 iters; PE 262.8 near its 254.6 fp32 floor - 4 cyc/row,
p-state-ramped, and any hi/lo split scheme at equal precision also needs
>=4 cyc/row so fp32 is optimal; DMA-device 225 serialized at 360GB/s
aggregate; Pool ~215; ACT 92) vs 339985ns for the previous t2/m2/is_ge/mult
kernel; verified rel err 6.032e-3 (bit-identical mismatch stats to v1).
Known floors: DVE 2-tensor ops are 1 cyc/elem fp32 (2x_2p half-cycle mode
applies only to 1-tensor ops: TensorCopy/TensorScalar); the 9-cell compare
needs >=5 two-stream passes (coverage doubling argument); remaining e2e gap
is ~27.6us pipeline fill, mostly PE p-state ramp physics + load latency
(slicing iter 0 finer was tried and made the sim schedule worse).
"""

import numpy as np

B, C, H, W = 256, 2, 224, 398
N_CORES = 8
PLANES = B * C                    # 512
P_CORE = PLANES // N_CORES        # 64 planes per core
GRP = 4                           # planes per supertile
KS = 5
SIGMA = 2.0
THR = 0.3

# chunk geometry: (raw_row0, out_row0)
#  chunk 0: blur rows 0..112   (ext: [b0, b0..b112]),  raw rows 0..114
#  chunk 1: blur rows 111..223 (ext: [b111..b223, b223]), raw rows 109..223
CHUNKS = [(0, 0), (109, 112)]
KDIM = 115                        # raw input rows per chunk
MDIM = 114                        # ext blur rows per chunk (113 + 1 dup)
OUTR = 112                        # output rows per chunk
NEGPAD = -1.0e30                  # vm pad columns (acts as maxpool -inf pad)

_nc_cache = {}


def _gauss():
    x = np.arange(KS, dtype=np.float32) - np.float32((KS - 1) / 2.0)
    g = np.exp(np.float32(-0.5) * (x / np.float32(SIGMA)) ** 2).astype(np.float32)
    g = (g / g.sum()).astype(np.float32)
    return g


def _gmats():
    """lhsT matrices [2 chunks, 5 shifts, K=115, M=114] fp32, then packed
    to [115, 2*5*114] (partition dim = K first)."""
    g = _gauss()

    def refl(r):
        if r < 0:
            return -r
        if r >= H:
            return 2 * H - 2 - r
        return r

    out = np.zeros((2, KS, KDIM, MDIM), np.float32)
    for c, (raw0, _) in enumerate(CHUNKS):
        for m in range(MDIM):
            if c == 0:
                br = max(m - 1, 0)            # ext[0] duplicates blur row 0
            else:
                br = 111 + min(m, MDIM - 2)   # ext[113] duplicates blur row 223
            for i in range(KS):
                k = refl(br + i - 2) - raw0
                assert 0 <= k < KDIM
                for j in range(KS):
                    out[c, j, k, m] += g[i] * g[j]
    return np.ascontiguousarray(out.transpose(2, 0, 1, 3).reshape(KDIM, 2 * KS * MDIM))


def _gate_op():
    """Register (once) and return the custom DVE op GATE(a,b) = b * (b >= a).

    Follows the documented extension path for concourse.dve_ops (define a
    DveOp, give it an opcode row, list it in OPS/CUSTOM_DVE_SPECS) but does
    it at runtime since the repo tree is read-only. The uops sha is pinned
    from this process's own lower() output (same call compile() makes).
    """
    from concourse import dve_ops as dvo
    from concourse.dve_spec import Spec, Src0, Src1, lower
    from concourse.dve_uop import DveOpSpec

    name = "NMS_GATE_ANT"
    if name in dvo._SUB_OPCODE_FOR_NAME:
        return next(op for op in dvo.OPS if op.name == name)

    spec = Spec(
        body=Src1 * (Src1 >= Src0),
        reference=lambda in0, in1, s0, s1, imm2: (
            in1.astype(np.float32) * (in1 >= in0)
        ).astype(np.float32),
    )
    row = max(dvo._SUB_OPCODE_FOR_NAME.values()) + 1
    assert row < 0x20, "no free custom-DVE opcode rows"
    dvo._SUB_OPCODE_FOR_NAME[name] = row
    shas = {}
    for ver in ("v3", "v4"):
        uops = lower(spec, ver=ver)
        shas[ver] = DveOpSpec(name=name, opcode=row, uops=uops, rd1_en=True).sha(ver)
    op = dvo.DveOp(name, spec, subdim=False, uops_sha=shas)
    dvo.OPS.append(op)
    dvo.CUSTOM_DVE_SPECS[name] = spec
    return op


def _build():
    import concourse.bacc as bacc
    import concourse.tile as tile
    import concourse.mybir as mybir

    f32 = mybir.dt.float32
    bf16 = mybir.dt.bfloat16
    AOT = mybir.AluOpType
    ACT = mybir.ActivationFunctionType
    THRP = float(np.nextafter(np.float32(THR), np.float32(1.0)))
    GATE = _gate_op()

    nc = bacc.Bacc(trn_type="TRN2", target_bir_lowering=False, debug=False)
    x_t = nc.dram_tensor("x", [P_CORE, H, W], f32, kind="ExternalInput")
    g_t = nc.dram_tensor("g", [KDIM, 2 * KS * MDIM], f32, kind="ExternalInput")
    o_t = nc.dram_tensor("o", [P_CORE, H, W], bf16, kind="ExternalOutput")
    x_ap = x_t.ap()
    o_ap = o_t.ap()

    NGRP = P_CORE // GRP
    IT = [(grp, ci) for grp in range(NGRP) for ci in range(2)]
    LEAD = 2   # produce runs LEAD steps ahead of consume_mm (keeps PE fed)
    PRE = 1    # consume_chain trails consume_mm by PRE steps

    with tile.TileContext(nc) as tc:
        with tc.tile_pool(name="const", bufs=1) as constp, \
             tc.tile_pool(name="xin", bufs=LEAD + 3) as xinp, \
             tc.tile_pool(name="ssum", bufs=LEAD + 2) as ssump, \
             tc.tile_pool(name="work", bufs=3) as workp, \
             tc.tile_pool(name="ps", bufs=2, space="PSUM") as psp:
            gt = constp.tile([KDIM, 2 * KS * MDIM], f32, tag="g")
            nc.gpsimd.dma_start(out=gt[:], in_=g_t.ap())
            state = {}
            mmstate = {}

            def produce(it):
                grp, c = it
                raw0, _ = CHUNKS[c]
                p0 = grp * GRP
                # ---- load input tile (one batched DMA per group) ----
                xt = xinp.tile([KDIM, GRP, W], f32, tag="x")
                nc.sync.dma_start(
                    out=xt[:, :, :],
                    in_=x_ap[p0 : p0 + GRP, raw0 : raw0 + KDIM, :].transpose(
                        [1, 0, 2]
                    ),
                )
                # ---- shifted-sum tiles (exact fp32): s1[c]=x[c-1]+x[c+1],
                # s2[c]=x[c-2]+x[c+2]; horizontal reflect folds into the edge
                # columns as 2*x[k] (ACT scale-copies) or interior pairs.
                seng = nc.vector if IT.index(it) < 2 else nc.gpsimd
                s1 = ssump.tile([KDIM, GRP, W], f32, tag="s1", name=f"s1_{grp}_{c}")
                seng.tensor_tensor(
                    s1[:, :, 1 : W - 1], xt[:, :, 0 : W - 2], xt[:, :, 2:W], AOT.add
                )
                s2 = ssump.tile([KDIM, GRP, W], f32, tag="s2", name=f"s2_{grp}_{c}")
                seng.tensor_tensor(
                    s2[:, :, 2 : W - 2], xt[:, :, 0 : W - 4], xt[:, :, 4:W], AOT.add
                )
                state[it] = (xt, s1, s2)

            def consume_mm(it):
                grp, c = it
                raw0, out0 = CHUNKS[c]
                p0 = grp * GRP
                xt, s1, s2 = state[it]
                first = grp == 0
                last = grp == NGRP - 1
                # reflect edge columns of s1/s2, emitted here (not in
                # produce) so no engine queue ever waits on a future load
                nc.scalar.activation(s1[:, :, 0:1], xt[:, :, 1:2], ACT.Copy, scale=2.0)
                nc.scalar.activation(
                    s1[:, :, W - 1 : W], xt[:, :, W - 2 : W - 1], ACT.Copy, scale=2.0
                )
                nc.scalar.activation(s2[:, :, 0:1], xt[:, :, 2:3], ACT.Copy, scale=2.0)
                nc.scalar.activation(
                    s2[:, :, W - 1 : W], xt[:, :, W - 3 : W - 2], ACT.Copy, scale=2.0
                )
                nc.gpsimd.tensor_tensor(
                    s2[:, :, 1:2], xt[:, :, 1:2], xt[:, :, 3:4], AOT.add
                )
                nc.gpsimd.tensor_tensor(
                    s2[:, :, W - 2 : W - 1], xt[:, :, W - 4 : W - 3],
                    xt[:, :, W - 2 : W - 1], AOT.add,
                )

                # ---- full separable blur on PE: 3 accumulating matmuls ----
                pss = [
                    psp.tile([MDIM, 512], f32, tag=f"p{i}", name=f"ps_{grp}_{c}_{i}")
                    for i in range(GRP)
                ]
                # j=2 (center, no s1/s2 dependency) first for overlap: the
                # center matmuls need only xt, covering the s1/s2 Pool latency.
                order = [(j, i) for j in (2, 1, 0) for i in range(GRP)]
                for j, i in order:
                    term = (2, 1, 0).index(j)
                    lhs = gt[:, (c * KS + j) * MDIM : (c * KS + j + 1) * MDIM]
                    if j == 2:
                        rhs = xt[:, i, :]
                    elif j == 1:
                        rhs = s1[:, i, :]
                    else:
                        rhs = s2[:, i, :]
                    nc.tensor.matmul(
                        out=pss[i][:, 0:W],
                        lhsT=lhs,
                        rhs=rhs,
                        start=(term == 0),
                        stop=(term == 2),
                    )

                # ---- PSUM -> SBUF (ACT), plus shifted copies via DMA ----
                blur = workp.tile([MDIM, GRP, 400], f32, tag="blur")
                for i in range(GRP):
                    nc.scalar.copy(blur[:, i, 0:W], pss[i][:, 0:W])
                pl_slices = [slice(0, 2), slice(2, 4)]
                # blurdn[r] = ext[r+1]: the center row values for out row r
                # (also the t1 operand); blur2[r] = ext[r+2].
                blurdn = workp.tile([OUTR, GRP, 400], f32, tag="blurdn")
                blur2 = workp.tile([OUTR, GRP, 400], f32, tag="blur2")
                for sl in pl_slices:
                    nc.scalar.dma_start(
                        out=blurdn[:, sl, 0:W], in_=blur[1 : OUTR + 1, sl, 0:W]
                    )
                    nc.scalar.dma_start(
                        out=blur2[:, sl, 0:W], in_=blur[2 : OUTR + 2, sl, 0:W]
                    )
                # vm pad columns (survive all iterations of this tile buffer;
                # cheap Pool memsets, re-done per iter since pool bufs rotate)
                vm = workp.tile([OUTR, GRP, 400], f32, tag="vm")
                nc.gpsimd.memset(vm[:, :, 0:1], NEGPAD)
                nc.gpsimd.memset(vm[:, :, 399:400], NEGPAD)
                mmstate[it] = (blur, blurdn, blur2, vm)

            def consume_chain(it):
                grp, c = it
                raw0, out0 = CHUNKS[c]
                p0 = grp * GRP
                blur, blurdn, blur2, vm = mmstate.pop(it)
                state.pop(it)
                t1 = workp.tile([OUTR, GRP, 400], f32, tag="t1")
                outv = workp.tile([OUTR, GRP, W], bf16, tag="outv")
                # e1 reuses t1's buffer, e2 reuses blur2's (same-engine
                # in-order WAR: t1/blur2 are last read by the vm instruction,
                # e1/e2 written by later DVE instructions).
                e1 = t1
                e2 = blur2
                ch_slices = (
                    [slice(0, 2), slice(2, 4)] if it == IT[-1] else [slice(0, GRP)]
                )
                st_slices = [slice(0, 2), slice(2, 4)]
                for sl in ch_slices:
                    # ---- vertical 3x1 max (+ threshold fold) on DVE ----
                    nc.vector.scalar_tensor_tensor(
                        out=t1[:, sl, 0:W],
                        in0=blur[0:OUTR, sl, 0:W],
                        scalar=THRP,
                        in1=blurdn[:, sl, 0:W],
                        op0=AOT.max,
                        op1=AOT.max,
                    )
                    # vm data lands in cols 1..398 (pads at 0 and 399)
                    nc.vector.tensor_tensor(
                        vm[:, sl, 1 : W + 1], t1[:, sl, 0:W], blur2[:, sl, 0:W],
                        AOT.max,
                    )
                    # ---- horizontal max + NMS compare + select: 3 GATEs ----
                    nc.vector._custom_dve(
                        GATE, out=e1[:, sl, 0:W],
                        in0=vm[:, sl, 1 : W + 1], in1=blurdn[:, sl, 0:W],
                    )
                    nc.vector._custom_dve(
                        GATE, out=e2[:, sl, 0:W],
                        in0=vm[:, sl, 2 : W + 2], in1=e1[:, sl, 0:W],
                    )
                    nc.vector._custom_dve(
                        GATE, out=outv[:, sl, 0:W],
                        in0=vm[:, sl, 0:W], in1=e2[:, sl, 0:W],
                    )
                for st in (st_slices if len(ch_slices) == 1 else ch_slices):
                    nc.sync.dma_start(
                        out=o_ap[
                            p0 + st.start : p0 + st.stop, out0 : out0 + OUTR, :
                        ].transpose([1, 0, 2]),
                        in_=outv[:, st, 0:W],
                    )

            for step in range(len(IT) + LEAD + PRE):
                if LEAD <= step < len(IT) + LEAD:
                    consume_mm(IT[step - LEAD])
                if step < len(IT):
                    produce(IT[step])
                if step >= LEAD + PRE:
                    consume_chain(IT[step - LEAD - PRE])

    nc.compile()
    return nc


def _make_sharded():
    """Build the shard_map'd PJRT executable ONCE and cache it, so repeat
    kernel() calls skip jit re-tracing / recompilation (~6s/call)."""
    import jax
    from jax.sharding import Mesh, PartitionSpec, NamedSharding
    from jax.experimental.shard_map import shard_map
    import concourse.mybir as mybir
    from concourse import bass2jax
    from concourse.bass2jax import _bass_exec_p, install_neuronx_cc_hook

    nc = _nc_cache["nc"]
    install_neuronx_cc_hook()
    partition_name = nc.partition_id_tensor.name if nc.partition_id_tensor else None
    in_names, out_names, out_avals, zero_shapes = [], [], [], []
    for alloc in nc.m.functions[0].allocations:
        if not isinstance(alloc, mybir.MemoryLocationSet):
            continue
        name = alloc.memorylocations[0].name
        if alloc.kind == "ExternalInput":
            if name != partition_name:
                in_names.append(name)
        elif alloc.kind == "ExternalOutput":
            out_names.append(name)
            shape = tuple(alloc.tensor_shape)
            dtype = mybir.dt.np(alloc.dtype)
            out_avals.append(jax.core.ShapedArray(shape, dtype))
            zero_shapes.append((shape, dtype))
    n_params = len(in_names)
    n_outs = len(out_avals)
    all_in_names = list(in_names) + list(out_names)
    if partition_name is not None:
        all_in_names.append(partition_name)

    def _body(*args):
        operands = list(args)
        if partition_name is not None:
            operands.append(bass2jax.partition_id_tensor())
        return tuple(_bass_exec_p.bind(
            *operands,
            out_avals=tuple(out_avals),
            in_names=tuple(all_in_names),
            out_names=tuple(out_names),
            lowering_input_output_aliases=(),
            sim_require_finite=True,
            sim_require_nnan=True,
            nc=nc,
        ))

    devices = jax.devices()[:N_CORES]
    mesh = Mesh(np.asarray(devices), ("core",))
    sharded = jax.jit(
        shard_map(
            _body, mesh=mesh,
            in_specs=(PartitionSpec("core"),) * (n_params + n_outs),
            out_specs=(PartitionSpec("core"),) * len(out_names),
            check_rep=False,
        ),
        donate_argnums=tuple(range(n_params, n_params + n_outs)),
        keep_unused=True,
    )
    sh = NamedSharding(mesh, PartitionSpec("core"))
    return sharded, sh, in_names, out_names, zero_shapes


def kernel(hands_batch: np.ndarray) -> np.ndarray:
    import jax

    x = np.ascontiguousarray(np.asarray(hands_batch, dtype=np.float32))
    assert x.shape == (B, C, H, W)

    if "nc" not in _nc_cache:
        _nc_cache["nc"] = _build()
        _nc_cache["g"] = _gmats()
        _nc_cache["fn"] = _make_sharded()
    sharded, sh, in_names, out_names, zero_shapes = _nc_cache["fn"]
    gm = _nc_cache["g"]

    concat = {
        "x": x.reshape(N_CORES * P_CORE, H, W),
        "g": np.concatenate([gm] * N_CORES, axis=0),
    }
    args = [jax.device_put(concat[nm], sh) for nm in in_names]
    zeros = [
        jax.device_put(np.zeros((N_CORES * s[0], *s[1:]), d), sh)
        for (s, d) in zero_shapes
    ]
    outs = sharded(*args, *zeros)
    out = np.asarray(outs[out_names.index("o")]).astype(np.float32)
    return out.reshape(B, C, H, W)


if __name__ == "__main__":
    rng = np.random.default_rng(0)
    x = rng.random((B, C, H, W), dtype=np.float32)
    y = kernel(x)
    print("kernel ran, out shape", y.shape, "nonzero frac", (y != 0).mean())


# revision 72
# speedup vs baseline: 1.0162x; 1.0162x over previous
"""Trainium2 Bass kernel for nms_detection (GaussianBlur5x5 -> MaxPool3x3 -> peak NMS + threshold).

Contract: kernel(hands_batch) takes the FULL [256, 2, 224, 398] f32 input and
returns the FULL [256, 2, 224, 398] f32 peaks map. Internally data-parallel
over 8 NeuronCores: 512 planes -> 64 planes/core.

v2 of the previous 340us kernel: the entire horizontal-max + compare + select
tail (t2/m2/is_ge/mult = ~2.4 DVE passes + 0.6 Pool pass + edge stts in v1)
is replaced by a 3-instruction chain of ONE custom DVE op
  GATE(a, b) = b * (b >= a)
using the zero-annihilation trick: since vm >= THRP > 0 everywhere, a zeroed
value can never pass a later gate with a positive product, so
  e1 = GATE(vm,    bd)   (center-column test; bd = blur center row)
  e2 = GATE(vm>>1, e1)   (right neighbour; e1==0 rows stay 0)
  out = GATE(vm<<1, e2)  (left neighbour; emitted directly as bf16)
equals bd * [bd >= max3x3(blur), bd >= THRP] exactly (fp32 compares; the
only rounding is the final bf16 value store, same as v1). The vm tile keeps
-1e30 pad columns at both ends so no edge-column instructions are needed.

Per-core algorithm (plane = one [224, 398] image channel):
  - Rows live on SBUF partitions; H=224 splits into two chunks of 113 blur
    rows (+1 duplicated edge row -> M=114); 4 planes per supertile.
  - Blur: 3 accumulating fp32 matmuls per plane-chunk on the PE via gaussian
    symmetry gh=[a,b,c,b,a]: blur = (c*Gv)@x0 + (b*Gv)@s1 + (a*Gv)@s2 with
    s1=x[-1]+x[+1], s2=x[-2]+x[+2]. fp32 is mandatory: f32r/bf16/f16 matmuls
    measure ~11-bit mantissa on HW, flipping ~10k near-tie NMS compares (the
    2e-2 rel-err budget only allows ~500).
  - s1/s2 pre-adds on the Pool engine (gpsimd; HW ISA = add/mult/memset only,
    max/is_ge/stt/copy are rejected), ACT does PSUM->SBUF blur copies +
    reflect edge-column scale-copies.
  - vertical 3x1 max on DVE: two DMA-materialized partition-shifted copies
    (engines cannot read operands at mismatched partition offsets), then
    t1 = max(blur, blurdn, THRP) (stt) and vm = max(t1, blur2).
  - horizontal max + NMS compare + select: the 3 GATE instructions above.
Schedule: 3-stage software pipeline (emission order = tile-scheduler
priority): consume_mm(k) emitted BEFORE produce(k+LEAD) each step so the
shift-copy DMAs of iter k enqueue ahead of the input load of iter k+LEAD on
the serialized DMA device (the chain's critical dependency); produce leads
consume_mm by LEAD=2 (xin/ssum bufs=LEAD+2 breaks the Pool<->PE circular
wait through the s1/s2 buffer rotation that periodically stalled PE and
reset its p-state ramp; xin at LEAD+3=5 bufs gives the input loads extra
slack on the serialized DMA device, -701ns), chain trails by PRE=1. Shift copies issue from the
ACT queue (HWDGE), loads/stores from SP. DMA batched via transposed DRAM
access patterns. The first two iterations' s1/s2 run on the DVE (idle during
pipeline fill, and 2x Pool's rate), pulling the j=1/j=0 matmuls earlier;
the last iteration's chain+store is 2-plane-sliced so the final store
overlaps the final gates (drain); ALL stores issue as 2-plane slices for
finer interleave on the serialized DMA device (-200ns; 1-plane is worse).
TimelineSim: 306822ns e2e (DVE 275us busy at 97.5% occupancy = the wall:
t1/vm/3 gates x 32 iters; PE 262.8 near its 254.6 fp32 floor - 4 cyc/row,
p-state-ramped, and any hi/lo split scheme at equal precision also needs
>=4 cyc/row so fp32 is optimal; DMA-device 225 serialized at 360GB/s
aggregate; Pool ~215; ACT 92) vs 339985ns for the previous t2/m2/is_ge/mult
kernel; verified rel err 6.032e-3 (bit-identical mismatch stats to v1).
Known floors: DVE 2-tensor ops are 1 cyc/elem fp32 (2x_2p half-cycle mode
applies only to 1-tensor ops: TensorCopy/TensorScalar); the 9-cell compare
needs >=5 two-stream passes (coverage doubling argument); remaining e2e gap
is ~27.6us pipeline fill, mostly PE p-state ramp physics + load latency
(slicing iter 0 finer was tried and made the sim schedule worse).
"""

import numpy as np

B, C, H, W = 256, 2, 224, 398
N_CORES = 8
PLANES = B * C                    # 512
P_CORE = PLANES // N_CORES        # 64 planes per core
GRP = 4                           # planes per supertile
KS = 5
SIGMA = 2.0
THR = 0.3

# chunk geometry: (raw_row0, out_row0)
#  chunk 0: blur rows 0..112   (ext: [b0, b0..b112]),  raw rows 0..114
#  chunk 1: blur rows 111..223 (ext: [b111..b223, b223]), raw rows 109..223
CHUNKS = [(0, 0), (109, 112)]
KDIM = 115                        # raw input rows per chunk
MDIM = 114                        # ext blur rows per chunk (113 + 1 dup)
OUTR = 112                        # output rows per chunk
NEGPAD = -1.0e30                  # vm pad columns (acts as maxpool -inf pad)

_nc_cache = {}


def _gauss():
    x = np.arange(KS, dtype=np.float32) - np.float32((KS - 1) / 2.0)
    g = np.exp(np.float32(-0.5) * (x / np.float32(SIGMA)) ** 2).astype(np.float32)
    g = (g / g.sum()).astype(np.float32)
    return g


def _gmats():
    """lhsT matrices [2 chunks, 5 shifts, K=115, M=114] fp32, then packed
    to [115, 2*5*114] (partition dim = K first)."""
    g = _gauss()

    def refl(r):
        if r < 0:
            return -r
        if r >= H:
            return 2 * H - 2 - r
        return r

    out = np.zeros((2, KS, KDIM, MDIM), np.float32)
    for c, (raw0, _) in enumerate(CHUNKS):
        for m in range(MDIM):
            if c == 0:
                br = max(m - 1, 0)            # ext[0] duplicates blur row 0
            else:
                br = 111 + min(m, MDIM - 2)   # ext[113] duplicates blur row 223
            for i in range(KS):
                k = refl(br + i - 2) - raw0
                assert 0 <= k < KDIM
                for j in range(KS):
                    out[c, j, k, m] += g[i] * g[j]
    return np.ascontiguousarray(out.transpose(2, 0, 1, 3).reshape(KDIM, 2 * KS * MDIM))


def _gate_op():
    """Register (once) and return the custom DVE op GATE(a,b) = b * (b >= a).

    Follows the documented extension path for concourse.dve_ops (define a
    DveOp, give it an opcode row, list it in OPS/CUSTOM_DVE_SPECS) but does
    it at runtime since the repo tree is read-only. The uops sha is pinned
    from this process's own lower() output (same call compile() makes).
    """
    from concourse import dve_ops as dvo
    from concourse.dve_spec import Spec, Src0, Src1, lower
    from concourse.dve_uop import DveOpSpec

    name = "NMS_GATE_ANT"
    if name in dvo._SUB_OPCODE_FOR_NAME:
        return next(op for op in dvo.OPS if op.name == name)

    spec = Spec(
        body=Src1 * (Src1 >= Src0),
        reference=lambda in0, in1, s0, s1, imm2: (
            in1.astype(np.float32) * (in1 >= in0)
        ).astype(np.float32),
    )
    row = max(dvo._SUB_OPCODE_FOR_NAME.values()) + 1
    assert row < 0x20, "no free custom-DVE opcode rows"
    dvo._SUB_OPCODE_FOR_NAME[name] = row
    shas = {}
    for ver in ("v3", "v4"):
        uops = lower(spec, ver=ver)
        shas[ver] = DveOpSpec(name=name, opcode=row, uops=uops, rd1_en=True).sha(ver)
    op = dvo.DveOp(name, spec, subdim=False, uops_sha=shas)
    dvo.OPS.append(op)
    dvo.CUSTOM_DVE_SPECS[name] = spec
    return op


def _build():
    import concourse.bacc as bacc
    import concourse.tile as tile
    import concourse.mybir as mybir

    f32 = mybir.dt.float32
    bf16 = mybir.dt.bfloat16
    AOT = mybir.AluOpType
    ACT = mybir.ActivationFunctionType
    THRP = float(np.nextafter(np.float32(THR), np.float32(1.0)))
    GATE = _gate_op()

    nc = bacc.Bacc(trn_type="TRN2", target_bir_lowering=False, debug=False)
    x_t = nc.dram_tensor("x", [P_CORE, H, W], f32, kind="ExternalInput")
    g_t = nc.dram_tensor("g", [KDIM, 2 * KS * MDIM], f32, kind="ExternalInput")
    o_t = nc.dram_tensor("o", [P_CORE, H, W], bf16, kind="ExternalOutput")
    x_ap = x_t.ap()
    o_ap = o_t.ap()

    NGRP = P_CORE // GRP
    IT = [(grp, ci) for grp in range(NGRP) for ci in range(2)]
    LEAD = 2   # produce runs LEAD steps ahead of consume_mm (keeps PE fed)
    PRE = 1    # consume_chain trails consume_mm by PRE steps

    with tile.TileContext(nc) as tc:
        with tc.tile_pool(name="const", bufs=1) as constp, \
             tc.tile_pool(name="xin", bufs=LEAD + 3) as xinp, \
             tc.tile_pool(name="ssum", bufs=LEAD + 2) as ssump, \
             tc.tile_pool(name="work", bufs=3) as workp, \
             tc.tile_pool(name="ps", bufs=2, space="PSUM") as psp:
            gt = constp.tile([KDIM, 2 * KS * MDIM], f32, tag="g")
            nc.gpsimd.dma_start(out=gt[:], in_=g_t.ap())
            state = {}
            mmstate = {}

            def produce(it):
                grp, c = it
                raw0, _ = CHUNKS[c]
                p0 = grp * GRP
                # ---- load input tile (one batched DMA per group) ----
                xt = xinp.tile([KDIM, GRP, W], f32, tag="x")
                ld_slices = (
                    [slice(i, i + 1) for i in range(GRP)]
                    if IT.index(it) < 2
                    else [slice(0, GRP)]
                )
                for ld in ld_slices:
                    nc.sync.dma_start(
                        out=xt[:, ld, :],
                        in_=x_ap[
                            p0 + ld.start : p0 + ld.stop, raw0 : raw0 + KDIM, :
                        ].transpose([1, 0, 2]),
                    )
                # ---- shifted-sum tiles (exact fp32): s1[c]=x[c-1]+x[c+1],
                # s2[c]=x[c-2]+x[c+2]; horizontal reflect folds into the edge
                # columns as 2*x[k] (ACT scale-copies) or interior pairs.
                seng = nc.vector if IT.index(it) < 2 else nc.gpsimd
                s1 = ssump.tile([KDIM, GRP, W], f32, tag="s1", name=f"s1_{grp}_{c}")
                s2 = ssump.tile([KDIM, GRP, W], f32, tag="s2", name=f"s2_{grp}_{c}")
                for ld in ld_slices:
                    seng.tensor_tensor(
                        s1[:, ld, 1 : W - 1], xt[:, ld, 0 : W - 2], xt[:, ld, 2:W],
                        AOT.add,
                    )
                    seng.tensor_tensor(
                        s2[:, ld, 2 : W - 2], xt[:, ld, 0 : W - 4], xt[:, ld, 4:W],
                        AOT.add,
                    )
                state[it] = (xt, s1, s2)

            def consume_mm(it):
                grp, c = it
                raw0, out0 = CHUNKS[c]
                p0 = grp * GRP
                xt, s1, s2 = state[it]
                first = grp == 0
                last = grp == NGRP - 1
                # reflect edge columns of s1/s2, emitted here (not in
                # produce) so no engine queue ever waits on a future load
                nc.scalar.activation(s1[:, :, 0:1], xt[:, :, 1:2], ACT.Copy, scale=2.0)
                nc.scalar.activation(
                    s1[:, :, W - 1 : W], xt[:, :, W - 2 : W - 1], ACT.Copy, scale=2.0
                )
                nc.scalar.activation(s2[:, :, 0:1], xt[:, :, 2:3], ACT.Copy, scale=2.0)
                nc.scalar.activation(
                    s2[:, :, W - 1 : W], xt[:, :, W - 3 : W - 2], ACT.Copy, scale=2.0
                )
                nc.gpsimd.tensor_tensor(
                    s2[:, :, 1:2], xt[:, :, 1:2], xt[:, :, 3:4], AOT.add
                )
                nc.gpsimd.tensor_tensor(
                    s2[:, :, W - 2 : W - 1], xt[:, :, W - 4 : W - 3],
                    xt[:, :, W - 2 : W - 1], AOT.add,
                )

                # ---- full separable blur on PE: 3 accumulating matmuls ----
                pss = [
                    psp.tile([MDIM, 512], f32, tag=f"p{i}", name=f"ps_{grp}_{c}_{i}")
                    for i in range(GRP)
                ]
                # j=2 (center, no s1/s2 dependency) first for overlap: the
                # center matmuls need only xt, covering the s1/s2 Pool latency.
                order = [(j, i) for j in (2, 1, 0) for i in range(GRP)]
                for j, i in order:
                    term = (2, 1, 0).index(j)
                    lhs = gt[:, (c * KS + j) * MDIM : (c * KS + j + 1) * MDIM]
                    if j == 2:
                        rhs = xt[:, i, :]
                    elif j == 1:
                        rhs = s1[:, i, :]
                    else:
                        rhs = s2[:, i, :]
                    nc.tensor.matmul(
                        out=pss[i][:, 0:W],
                        lhsT=lhs,
                        rhs=rhs,
                        start=(term == 0),
                        stop=(term == 2),
                    )

                # ---- PSUM -> SBUF (ACT), plus shifted copies via DMA ----
                blur = workp.tile([MDIM, GRP, 400], f32, tag="blur")
                for i in range(GRP):
                    nc.scalar.copy(blur[:, i, 0:W], pss[i][:, 0:W])
                pl_slices = [slice(0, 2), slice(2, 4)]
                # blurdn[r] = ext[r+1]: the center row values for out row r
                # (also the t1 operand); blur2[r] = ext[r+2].
                blurdn = workp.tile([OUTR, GRP, 400], f32, tag="blurdn")
                blur2 = workp.tile([OUTR, GRP, 400], f32, tag="blur2")
                for sl in pl_slices:
                    nc.scalar.dma_start(
                        out=blurdn[:, sl, 0:W], in_=blur[1 : OUTR + 1, sl, 0:W]
                    )
                    nc.scalar.dma_start(
                        out=blur2[:, sl, 0:W], in_=blur[2 : OUTR + 2, sl, 0:W]
                    )
                # vm pad columns (survive all iterations of this tile buffer;
                # cheap Pool memsets, re-done per iter since pool bufs rotate)
                vm = workp.tile([OUTR, GRP, 400], f32, tag="vm")
                nc.gpsimd.memset(vm[:, :, 0:1], NEGPAD)
                nc.gpsimd.memset(vm[:, :, 399:400], NEGPAD)
                mmstate[it] = (blur, blurdn, blur2, vm)

            def consume_chain(it):
                grp, c = it
                raw0, out0 = CHUNKS[c]
                p0 = grp * GRP
                blur, blurdn, blur2, vm = mmstate.pop(it)
                state.pop(it)
                t1 = workp.tile([OUTR, GRP, 400], f32, tag="t1")
                outv = workp.tile([OUTR, GRP, W], bf16, tag="outv")
                # e1 reuses t1's buffer, e2 reuses blur2's (same-engine
                # in-order WAR: t1/blur2 are last read by the vm instruction,
                # e1/e2 written by later DVE instructions).
                e1 = t1
                e2 = blur2
                ch_slices = (
                    [slice(0, 2), slice(2, 4)] if it == IT[-1] else [slice(0, GRP)]
                )
                st_slices = [slice(0, 2), slice(2, 4)]
                for sl in ch_slices:
                    # ---- vertical 3x1 max (+ threshold fold) on DVE ----
                    nc.vector.scalar_tensor_tensor(
                        out=t1[:, sl, 0:W],
                        in0=blur[0:OUTR, sl, 0:W],
                        scalar=THRP,
                        in1=blurdn[:, sl, 0:W],
                        op0=AOT.max,
                        op1=AOT.max,
                    )
                    # vm data lands in cols 1..398 (pads at 0 and 399)
                    nc.vector.tensor_tensor(
                        vm[:, sl, 1 : W + 1], t1[:, sl, 0:W], blur2[:, sl, 0:W],
                        AOT.max,
                    )
                    # ---- horizontal max + NMS compare + select: 3 GATEs ----
                    nc.vector._custom_dve(
                        GATE, out=e1[:, sl, 0:W],
                        in0=vm[:, sl, 1 : W + 1], in1=blurdn[:, sl, 0:W],
                    )
                    nc.vector._custom_dve(
                        GATE, out=e2[:, sl, 0:W],
                        in0=vm[:, sl, 2 : W + 2], in1=e1[:, sl, 0:W],
                    )
                    nc.vector._custom_dve(
                        GATE, out=outv[:, sl, 0:W],
                        in0=vm[:, sl, 0:W], in1=e2[:, sl, 0:W],
                    )
                for st in (st_slices if len(ch_slices) == 1 else ch_slices):
                    nc.sync.dma_start(
                        out=o_ap[
                            p0 + st.start : p0 + st.stop, out0 : out0 + OUTR, :
                        ].transpose([1, 0, 2]),
                        in_=outv[:, st, 0:W],
                    )

            for step in range(len(IT) + LEAD + PRE):
                if LEAD <= step < len(IT) + LEAD:
                    consume_mm(IT[step - LEAD])
                if step < len(IT):
                    produce(IT[step])
                if step >= LEAD + PRE:
                    consume_chain(IT[step - LEAD - PRE])

    nc.compile()
    return nc


def _make_sharded():
    """Build the shard_map'd PJRT executable ONCE and cache it, so repeat
    kernel() calls skip jit re-tracing / recompilation (~6s/call)."""
    import jax
    from jax.sharding import Mesh, PartitionSpec, NamedSharding
    from jax.experimental.shard_map import shard_map
    import concourse.mybir as mybir
    from concourse import bass2jax
    from concourse.bass2jax import _bass_exec_p, install_neuronx_cc_hook

    nc = _nc_cache["nc"]
    install_neuronx_cc_hook()
    partition_name = nc.partition_id_tensor.name if nc.partition_id_tensor else None
    in_names, out_names, out_avals, zero_shapes = [], [], [], []
    for alloc in nc.m.functions[0].allocations:
        if not isinstance(alloc, mybir.MemoryLocationSet):
            continue
        name = alloc.memorylocations[0].name
        if alloc.kind == "ExternalInput":
            if name != partition_name:
                in_names.append(name)
        elif alloc.kind == "ExternalOutput":
            out_names.append(name)
            shape = tuple(alloc.tensor_shape)
            dtype = mybir.dt.np(alloc.dtype)
            out_avals.append(jax.core.ShapedArray(shape, dtype))
            zero_shapes.append((shape, dtype))
    n_params = len(in_names)
    n_outs = len(out_avals)
    all_in_names = list(in_names) + list(out_names)
    if partition_name is not None:
        all_in_names.append(partition_name)

    def _body(*args):
        operands = list(args)
        if partition_name is not None:
            operands.append(bass2jax.partition_id_tensor())
        return tuple(_bass_exec_p.bind(
            *operands,
            out_avals=tuple(out_avals),
            in_names=tuple(all_in_names),
            out_names=tuple(out_names),
            lowering_input_output_aliases=(),
            sim_require_finite=True,
            sim_require_nnan=True,
            nc=nc,
        ))

    devices = jax.devices()[:N_CORES]
    mesh = Mesh(np.asarray(devices), ("core",))
    sharded = jax.jit(
        shard_map(
            _body, mesh=mesh,
            in_specs=(PartitionSpec("core"),) * (n_params + n_outs),
            out_specs=(PartitionSpec("core"),) * len(out_names),
            check_rep=False,
        ),
        donate_argnums=tuple(range(n_params, n_params + n_outs)),
        keep_unused=True,
    )
    sh = NamedSharding(mesh, PartitionSpec("core"))
    return sharded, sh, in_names, out_names, zero_shapes


def kernel(hands_batch: np.ndarray) -> np.ndarray:
    import jax

    x = np.ascontiguousarray(np.asarray(hands_batch, dtype=np.float32))
    assert x.shape == (B, C, H, W)

    if "nc" not in _nc_cache:
        _nc_cache["nc"] = _build()
        _nc_cache["g"] = _gmats()
        _nc_cache["fn"] = _make_sharded()
    sharded, sh, in_names, out_names, zero_shapes = _nc_cache["fn"]
    gm = _nc_cache["g"]

    concat = {
        "x": x.reshape(N_CORES * P_CORE, H, W),
        "g": np.concatenate([gm] * N_CORES, axis=0),
    }
    args = [jax.device_put(concat[nm], sh) for nm in in_names]
    zeros = [
        jax.device_put(np.zeros((N_CORES * s[0], *s[1:]), d), sh)
        for (s, d) in zero_shapes
    ]
    outs = sharded(*args, *zeros)
    out = np.asarray(outs[out_names.index("o")]).astype(np.float32)
    return out.reshape(B, C, H, W)


if __name__ == "__main__":
    rng = np.random.default_rng(0)
    x = rng.random((B, C, H, W), dtype=np.float32)
    y = kernel(x)
    print("kernel ran, out shape", y.shape, "nonzero frac", (y != 0).mean())


# revision 79
# speedup vs baseline: 1.0207x; 1.0044x over previous
"""Trainium2 Bass kernel for nms_detection (GaussianBlur5x5 -> MaxPool3x3 -> peak NMS + threshold).

Contract: kernel(hands_batch) takes the FULL [256, 2, 224, 398] f32 input and
returns the FULL [256, 2, 224, 398] f32 peaks map. Internally data-parallel
over 8 NeuronCores: 512 planes -> 64 planes/core.

v2 of the previous 340us kernel: the entire horizontal-max + compare + select
tail (t2/m2/is_ge/mult = ~2.4 DVE passes + 0.6 Pool pass + edge stts in v1)
is replaced by a 3-instruction chain of ONE custom DVE op
  GATE(a, b) = b * (b >= a)
using the zero-annihilation trick: since vm >= THRP > 0 everywhere, a zeroed
value can never pass a later gate with a positive product, so
  e1 = GATE(vm,    bd)   (center-column test; bd = blur center row)
  e2 = GATE(vm>>1, e1)   (right neighbour; e1==0 rows stay 0)
  out = GATE(vm<<1, e2)  (left neighbour; emitted directly as bf16)
equals bd * [bd >= max3x3(blur), bd >= THRP] exactly (fp32 compares; the
only rounding is the final bf16 value store, same as v1). The vm tile keeps
-1e30 pad columns at both ends so no edge-column instructions are needed.

Per-core algorithm (plane = one [224, 398] image channel):
  - Rows live on SBUF partitions; H=224 splits into two chunks of 113 blur
    rows (+1 duplicated edge row -> M=114); 4 planes per supertile.
  - Blur: 3 accumulating fp32 matmuls per plane-chunk on the PE via gaussian
    symmetry gh=[a,b,c,b,a]: blur = (c*Gv)@x0 + (b*Gv)@s1 + (a*Gv)@s2 with
    s1=x[-1]+x[+1], s2=x[-2]+x[+2]. fp32 is mandatory: f32r/bf16/f16 matmuls
    measure ~11-bit mantissa on HW, flipping ~10k near-tie NMS compares (the
    2e-2 rel-err budget only allows ~500).
  - s1/s2 pre-adds on the Pool engine (gpsimd; HW ISA = add/mult/memset only,
    max/is_ge/stt/copy are rejected), ACT does PSUM->SBUF blur copies +
    reflect edge-column scale-copies.
  - vertical 3x1 max on DVE: two DMA-materialized partition-shifted copies
    (engines cannot read operands at mismatched partition offsets), then
    t1 = max(blur, blurdn, THRP) (stt) and vm = max(t1, blur2).
  - horizontal max + NMS compare + select: the 3 GATE instructions above.
Schedule: 3-stage software pipeline (emission order = tile-scheduler
priority): consume_mm(k) emitted BEFORE produce(k+LEAD) each step so the
shift-copy DMAs of iter k enqueue ahead of the input load of iter k+LEAD on
the serialized DMA device (the chain's critical dependency); produce leads
consume_mm by LEAD=2 (xin/ssum bufs=LEAD+2 breaks the Pool<->PE circular
wait through the s1/s2 buffer rotation that periodically stalled PE and
reset its p-state ramp; xin at LEAD+3=5 bufs gives the input loads extra
slack on the serialized DMA device, -701ns), chain trails by PRE=1. Shift copies issue from the
ACT queue (HWDGE), loads/stores from SP. DMA batched via transposed DRAM
access patterns. The first two iterations' s1/s2 run on the DVE (idle during
pipeline fill, and 2x Pool's rate), pulling the j=1/j=0 matmuls earlier;
iters 0-1 load x in 1-PLANE DMA slices (range-level tile deps let each
plane's matmul start as soon as its slice lands, cascading the fill ~4.5us
earlier) and iters 0-3 run the DVE chain 2-plane-sliced (starts on planes
0-1 before 2-3's shifts land);
the last iteration's chain+store is 2-plane-sliced so the final store
overlaps the final gates (drain); ALL stores issue as 2-plane slices for
finer interleave on the serialized DMA device (-200ns; 1-plane is worse).
TimelineSim: 300597ns e2e (DVE 275us busy at 97.5% occupancy = the wall:
t1/vm/3 gates x 32 iters; PE 262.8 near its 254.6 fp32 floor - 4 cyc/row,
p-state-ramped, and any hi/lo split scheme at equal precision also needs
>=4 cyc/row so fp32 is optimal; DMA-device 225 serialized at 360GB/s
aggregate; Pool ~215; ACT 92) vs 339985ns for the previous t2/m2/is_ge/mult
kernel; verified rel err 6.032e-3 (bit-identical mismatch stats to v1).
Known floors: DVE 2-tensor ops are 1 cyc/elem fp32 (2x_2p half-cycle mode
applies only to 1-tensor ops: TensorCopy/TensorScalar); the 9-cell compare
needs >=5 two-stream passes (coverage doubling argument); remaining e2e gap
is ~27.6us pipeline fill, mostly PE p-state ramp physics + load latency
(slicing iter 0 finer was tried and made the sim schedule worse).
"""

import numpy as np

B, C, H, W = 256, 2, 224, 398
N_CORES = 8
PLANES = B * C                    # 512
P_CORE = PLANES // N_CORES        # 64 planes per core
GRP = 4                           # planes per supertile
KS = 5
SIGMA = 2.0
THR = 0.3

# chunk geometry: (raw_row0, out_row0)
#  chunk 0: blur rows 0..112   (ext: [b0, b0..b112]),  raw rows 0..114
#  chunk 1: blur rows 111..223 (ext: [b111..b223, b223]), raw rows 109..223
CHUNKS = [(0, 0), (109, 112)]
KDIM = 115                        # raw input rows per chunk
MDIM = 114                        # ext blur rows per chunk (113 + 1 dup)
OUTR = 112                        # output rows per chunk
NEGPAD = -1.0e30                  # vm pad columns (acts as maxpool -inf pad)

_nc_cache = {}


def _gauss():
    x = np.arange(KS, dtype=np.float32) - np.float32((KS - 1) / 2.0)
    g = np.exp(np.float32(-0.5) * (x / np.float32(SIGMA)) ** 2).astype(np.float32)
    g = (g / g.sum()).astype(np.float32)
    return g


def _gmats():
    """lhsT matrices [2 chunks, 5 shifts, K=115, M=114] fp32, then packed
    to [115, 2*5*114] (partition dim = K first)."""
    g = _gauss()

    def refl(r):
        if r < 0:
            return -r
        if r >= H:
            return 2 * H - 2 - r
        return r

    out = np.zeros((2, KS, KDIM, MDIM), np.float32)
    for c, (raw0, _) in enumerate(CHUNKS):
        for m in range(MDIM):
            if c == 0:
                br = max(m - 1, 0)            # ext[0] duplicates blur row 0
            else:
                br = 111 + min(m, MDIM - 2)   # ext[113] duplicates blur row 223
            for i in range(KS):
                k = refl(br + i - 2) - raw0
                assert 0 <= k < KDIM
                for j in range(KS):
                    out[c, j, k, m] += g[i] * g[j]
    return np.ascontiguousarray(out.transpose(2, 0, 1, 3).reshape(KDIM, 2 * KS * MDIM))


def _gate_op():
    """Register (once) and return the custom DVE op GATE(a,b) = b * (b >= a).

    Follows the documented extension path for concourse.dve_ops (define a
    DveOp, give it an opcode row, list it in OPS/CUSTOM_DVE_SPECS) but does
    it at runtime since the repo tree is read-only. The uops sha is pinned
    from this process's own lower() output (same call compile() makes).
    """
    from concourse import dve_ops as dvo
    from concourse.dve_spec import Spec, Src0, Src1, lower
    from concourse.dve_uop import DveOpSpec

    name = "NMS_GATE_ANT"
    if name in dvo._SUB_OPCODE_FOR_NAME:
        return next(op for op in dvo.OPS if op.name == name)

    spec = Spec(
        body=Src1 * (Src1 >= Src0),
        reference=lambda in0, in1, s0, s1, imm2: (
            in1.astype(np.float32) * (in1 >= in0)
        ).astype(np.float32),
    )
    row = max(dvo._SUB_OPCODE_FOR_NAME.values()) + 1
    assert row < 0x20, "no free custom-DVE opcode rows"
    dvo._SUB_OPCODE_FOR_NAME[name] = row
    shas = {}
    for ver in ("v3", "v4"):
        uops = lower(spec, ver=ver)
        shas[ver] = DveOpSpec(name=name, opcode=row, uops=uops, rd1_en=True).sha(ver)
    op = dvo.DveOp(name, spec, subdim=False, uops_sha=shas)
    dvo.OPS.append(op)
    dvo.CUSTOM_DVE_SPECS[name] = spec
    return op


def _build():
    import concourse.bacc as bacc
    import concourse.tile as tile
    import concourse.mybir as mybir

    f32 = mybir.dt.float32
    bf16 = mybir.dt.bfloat16
    AOT = mybir.AluOpType
    ACT = mybir.ActivationFunctionType
    THRP = float(np.nextafter(np.float32(THR), np.float32(1.0)))
    GATE = _gate_op()

    nc = bacc.Bacc(trn_type="TRN2", target_bir_lowering=False, debug=False)
    x_t = nc.dram_tensor("x", [P_CORE, H, W], f32, kind="ExternalInput")
    g_t = nc.dram_tensor("g", [KDIM, 2 * KS * MDIM], f32, kind="ExternalInput")
    o_t = nc.dram_tensor("o", [P_CORE, H, W], bf16, kind="ExternalOutput")
    x_ap = x_t.ap()
    o_ap = o_t.ap()

    NGRP = P_CORE // GRP
    IT = [(grp, ci) for grp in range(NGRP) for ci in range(2)]
    LEAD = 2   # produce runs LEAD steps ahead of consume_mm (keeps PE fed)
    PRE = 1    # consume_chain trails consume_mm by PRE steps

    with tile.TileContext(nc) as tc:
        with tc.tile_pool(name="const", bufs=1) as constp, \
             tc.tile_pool(name="xin", bufs=LEAD + 3) as xinp, \
             tc.tile_pool(name="ssum", bufs=LEAD + 2) as ssump, \
             tc.tile_pool(name="work", bufs=3) as workp, \
             tc.tile_pool(name="ps", bufs=2, space="PSUM") as psp:
            gt = constp.tile([KDIM, 2 * KS * MDIM], f32, tag="g")
            nc.gpsimd.dma_start(out=gt[:], in_=g_t.ap())
            state = {}
            mmstate = {}

            def produce(it):
                grp, c = it
                raw0, _ = CHUNKS[c]
                p0 = grp * GRP
                # ---- load input tile (one batched DMA per group) ----
                xt = xinp.tile([KDIM, GRP, W], f32, tag="x")
                ld_slices = (
                    [slice(i, i + 1) for i in range(GRP)]
                    if IT.index(it) < 2
                    else [slice(0, GRP)]
                )
                for ld in ld_slices:
                    nc.sync.dma_start(
                        out=xt[:, ld, :],
                        in_=x_ap[
                            p0 + ld.start : p0 + ld.stop, raw0 : raw0 + KDIM, :
                        ].transpose([1, 0, 2]),
                    )
                # ---- shifted-sum tiles (exact fp32): s1[c]=x[c-1]+x[c+1],
                # s2[c]=x[c-2]+x[c+2]; horizontal reflect folds into the edge
                # columns as 2*x[k] (ACT scale-copies) or interior pairs.
                seng = nc.vector if IT.index(it) < 2 else nc.gpsimd
                s1 = ssump.tile([KDIM, GRP, W], f32, tag="s1", name=f"s1_{grp}_{c}")
                s2 = ssump.tile([KDIM, GRP, W], f32, tag="s2", name=f"s2_{grp}_{c}")
                for ld in ld_slices:
                    seng.tensor_tensor(
                        s1[:, ld, 1 : W - 1], xt[:, ld, 0 : W - 2], xt[:, ld, 2:W],
                        AOT.add,
                    )
                    seng.tensor_tensor(
                        s2[:, ld, 2 : W - 2], xt[:, ld, 0 : W - 4], xt[:, ld, 4:W],
                        AOT.add,
                    )
                state[it] = (xt, s1, s2)

            def consume_mm(it):
                grp, c = it
                raw0, out0 = CHUNKS[c]
                p0 = grp * GRP
                xt, s1, s2 = state[it]
                first = grp == 0
                last = grp == NGRP - 1
                # reflect edge columns of s1/s2, emitted here (not in
                # produce) so no engine queue ever waits on a future load
                nc.scalar.activation(s1[:, :, 0:1], xt[:, :, 1:2], ACT.Copy, scale=2.0)
                nc.scalar.activation(
                    s1[:, :, W - 1 : W], xt[:, :, W - 2 : W - 1], ACT.Copy, scale=2.0
                )
                nc.scalar.activation(s2[:, :, 0:1], xt[:, :, 2:3], ACT.Copy, scale=2.0)
                nc.scalar.activation(
                    s2[:, :, W - 1 : W], xt[:, :, W - 3 : W - 2], ACT.Copy, scale=2.0
                )
                nc.gpsimd.tensor_tensor(
                    s2[:, :, 1:2], xt[:, :, 1:2], xt[:, :, 3:4], AOT.add
                )
                nc.gpsimd.tensor_tensor(
                    s2[:, :, W - 2 : W - 1], xt[:, :, W - 4 : W - 3],
                    xt[:, :, W - 2 : W - 1], AOT.add,
                )

                # ---- full separable blur on PE: 3 accumulating matmuls ----
                pss = [
                    psp.tile([MDIM, 512], f32, tag=f"p{i}", name=f"ps_{grp}_{c}_{i}")
                    for i in range(GRP)
                ]
                # j=2 (center, no s1/s2 dependency) first for overlap: the
                # center matmuls need only xt, covering the s1/s2 Pool latency.
                order = [(j, i) for j in (2, 1, 0) for i in range(GRP)]
                for j, i in order:
                    term = (2, 1, 0).index(j)
                    lhs = gt[:, (c * KS + j) * MDIM : (c * KS + j + 1) * MDIM]
                    if j == 2:
                        rhs = xt[:, i, :]
                    elif j == 1:
                        rhs = s1[:, i, :]
                    else:
                        rhs = s2[:, i, :]
                    nc.tensor.matmul(
                        out=pss[i][:, 0:W],
                        lhsT=lhs,
                        rhs=rhs,
                        start=(term == 0),
                        stop=(term == 2),
                    )

                # ---- PSUM -> SBUF (ACT), plus shifted copies via DMA ----
                blur = workp.tile([MDIM, GRP, 400], f32, tag="blur")
                for i in range(GRP):
                    nc.scalar.copy(blur[:, i, 0:W], pss[i][:, 0:W])
                pl_slices = [slice(0, 2), slice(2, 4)]
                # blurdn[r] = ext[r+1]: the center row values for out row r
                # (also the t1 operand); blur2[r] = ext[r+2].
                blurdn = workp.tile([OUTR, GRP, 400], f32, tag="blurdn")
                blur2 = workp.tile([OUTR, GRP, 400], f32, tag="blur2")
                for sl in pl_slices:
                    nc.scalar.dma_start(
                        out=blurdn[:, sl, 0:W], in_=blur[1 : OUTR + 1, sl, 0:W]
                    )
                    nc.scalar.dma_start(
                        out=blur2[:, sl, 0:W], in_=blur[2 : OUTR + 2, sl, 0:W]
                    )
                # vm pad columns (survive all iterations of this tile buffer;
                # cheap Pool memsets, re-done per iter since pool bufs rotate)
                vm = workp.tile([OUTR, GRP, 400], f32, tag="vm")
                nc.gpsimd.memset(vm[:, :, 0:1], NEGPAD)
                nc.gpsimd.memset(vm[:, :, 399:400], NEGPAD)
                mmstate[it] = (blur, blurdn, blur2, vm)

            def consume_chain(it):
                grp, c = it
                raw0, out0 = CHUNKS[c]
                p0 = grp * GRP
                blur, blurdn, blur2, vm = mmstate.pop(it)
                state.pop(it)
                t1 = workp.tile([OUTR, GRP, 400], f32, tag="t1")
                outv = workp.tile([OUTR, GRP, W], bf16, tag="outv")
                # e1 reuses t1's buffer, e2 reuses blur2's (same-engine
                # in-order WAR: t1/blur2 are last read by the vm instruction,
                # e1/e2 written by later DVE instructions).
                e1 = t1
                e2 = blur2
                ch_slices = (
                    [slice(0, 2), slice(2, 4)]
                    if it in IT[:4] or it == IT[-1]
                    else [slice(0, GRP)]
                )
                st_slices = [slice(0, 2), slice(2, 4)]
                for sl in ch_slices:
                    # ---- vertical 3x1 max (+ threshold fold) on DVE ----
                    nc.vector.scalar_tensor_tensor(
                        out=t1[:, sl, 0:W],
                        in0=blur[0:OUTR, sl, 0:W],
                        scalar=THRP,
                        in1=blurdn[:, sl, 0:W],
                        op0=AOT.max,
                        op1=AOT.max,
                    )
                    # vm data lands in cols 1..398 (pads at 0 and 399)
                    nc.vector.tensor_tensor(
                        vm[:, sl, 1 : W + 1], t1[:, sl, 0:W], blur2[:, sl, 0:W],
                        AOT.max,
                    )
                    # ---- horizontal max + NMS compare + select: 3 GATEs ----
                    nc.vector._custom_dve(
                        GATE, out=e1[:, sl, 0:W],
                        in0=vm[:, sl, 1 : W + 1], in1=blurdn[:, sl, 0:W],
                    )
                    nc.vector._custom_dve(
                        GATE, out=e2[:, sl, 0:W],
                        in0=vm[:, sl, 2 : W + 2], in1=e1[:, sl, 0:W],
                    )
                    nc.vector._custom_dve(
                        GATE, out=outv[:, sl, 0:W],
                        in0=vm[:, sl, 0:W], in1=e2[:, sl, 0:W],
                    )
                for st in (st_slices if len(ch_slices) == 1 else ch_slices):
                    nc.sync.dma_start(
                        out=o_ap[
                            p0 + st.start : p0 + st.stop, out0 : out0 + OUTR, :
                        ].transpose([1, 0, 2]),
                        in_=outv[:, st, 0:W],
                    )

            for step in range(len(IT) + LEAD + PRE):
                if LEAD <= step < len(IT) + LEAD:
                    consume_mm(IT[step - LEAD])
                if step < len(IT):
                    produce(IT[step])
                if step >= LEAD + PRE:
                    consume_chain(IT[step - LEAD - PRE])

    nc.compile()
    return nc


def _make_sharded():
    """Build the shard_map'd PJRT executable ONCE and cache it, so repeat
    kernel() calls skip jit re-tracing / recompilation (~6s/call)."""
    import jax
    from jax.sharding import Mesh, PartitionSpec, NamedSharding
    from jax.experimental.shard_map import shard_map
    import concourse.mybir as mybir
    from concourse import bass2jax
    from concourse.bass2jax import _bass_exec_p, install_neuronx_cc_hook

    nc = _nc_cache["nc"]
    install_neuronx_cc_hook()
    partition_name = nc.partition_id_tensor.name if nc.partition_id_tensor else None
    in_names, out_names, out_avals, zero_shapes = [], [], [], []
    for alloc in nc.m.functions[0].allocations:
        if not isinstance(alloc, mybir.MemoryLocationSet):
            continue
        name = alloc.memorylocations[0].name
        if alloc.kind == "ExternalInput":
            if name != partition_name:
                in_names.append(name)
        elif alloc.kind == "ExternalOutput":
            out_names.append(name)
            shape = tuple(alloc.tensor_shape)
            dtype = mybir.dt.np(alloc.dtype)
            out_avals.append(jax.core.ShapedArray(shape, dtype))
            zero_shapes.append((shape, dtype))
    n_params = len(in_names)
    n_outs = len(out_avals)
    all_in_names = list(in_names) + list(out_names)
    if partition_name is not None:
        all_in_names.append(partition_name)

    def _body(*args):
        operands = list(args)
        if partition_name is not None:
            operands.append(bass2jax.partition_id_tensor())
        return tuple(_bass_exec_p.bind(
            *operands,
            out_avals=tuple(out_avals),
            in_names=tuple(all_in_names),
            out_names=tuple(out_names),
            lowering_input_output_aliases=(),
            sim_require_finite=True,
            sim_require_nnan=True,
            nc=nc,
        ))

    devices = jax.devices()[:N_CORES]
    mesh = Mesh(np.asarray(devices), ("core",))
    sharded = jax.jit(
        shard_map(
            _body, mesh=mesh,
            in_specs=(PartitionSpec("core"),) * (n_params + n_outs),
            out_specs=(PartitionSpec("core"),) * len(out_names),
            check_rep=False,
        ),
        donate_argnums=tuple(range(n_params, n_params + n_outs)),
        keep_unused=True,
    )
    sh = NamedSharding(mesh, PartitionSpec("core"))
    return sharded, sh, in_names, out_names, zero_shapes


def kernel(hands_batch: np.ndarray) -> np.ndarray:
    import jax

    x = np.ascontiguousarray(np.asarray(hands_batch, dtype=np.float32))
    assert x.shape == (B, C, H, W)

    if "nc" not in _nc_cache:
        _nc_cache["nc"] = _build()
        _nc_cache["g"] = _gmats()
        _nc_cache["fn"] = _make_sharded()
    sharded, sh, in_names, out_names, zero_shapes = _nc_cache["fn"]
    gm = _nc_cache["g"]

    concat = {
        "x": x.reshape(N_CORES * P_CORE, H, W),
        "g": np.concatenate([gm] * N_CORES, axis=0),
    }
    args = [jax.device_put(concat[nm], sh) for nm in in_names]
    zeros = [
        jax.device_put(np.zeros((N_CORES * s[0], *s[1:]), d), sh)
        for (s, d) in zero_shapes
    ]
    outs = sharded(*args, *zeros)
    out = np.asarray(outs[out_names.index("o")]).astype(np.float32)
    return out.reshape(B, C, H, W)


if __name__ == "__main__":
    rng = np.random.default_rng(0)
    x = rng.random((B, C, H, W), dtype=np.float32)
    y = kernel(x)
    print("kernel ran, out shape", y.shape, "nonzero frac", (y != 0).mean())


# revision 86
# speedup vs baseline: 1.0254x; 1.0046x over previous
"""Trainium2 Bass kernel for nms_detection (GaussianBlur5x5 -> MaxPool3x3 -> peak NMS + threshold).

Contract: kernel(hands_batch) takes the FULL [256, 2, 224, 398] f32 input and
returns the FULL [256, 2, 224, 398] f32 peaks map. Internally data-parallel
over 8 NeuronCores: 512 planes -> 64 planes/core.

v2 of the previous 340us kernel: the entire horizontal-max + compare + select
tail (t2/m2/is_ge/mult = ~2.4 DVE passes + 0.6 Pool pass + edge stts in v1)
is replaced by a 3-instruction chain of ONE custom DVE op
  GATE(a, b) = b * (b >= a)
using the zero-annihilation trick: since vm >= THRP > 0 everywhere, a zeroed
value can never pass a later gate with a positive product, so
  e1 = GATE(vm,    bd)   (center-column test; bd = blur center row)
  e2 = GATE(vm>>1, e1)   (right neighbour; e1==0 rows stay 0)
  out = GATE(vm<<1, e2)  (left neighbour; emitted directly as bf16)
equals bd * [bd >= max3x3(blur), bd >= THRP] exactly (fp32 compares; the
only rounding is the final bf16 value store, same as v1). The vm tile keeps
-1e30 pad columns at both ends so no edge-column instructions are needed.

Per-core algorithm (plane = one [224, 398] image channel):
  - Rows live on SBUF partitions; H=224 splits into two chunks of 113 blur
    rows (+1 duplicated edge row -> M=114); 4 planes per supertile.
  - Blur: 3 accumulating fp32 matmuls per plane-chunk on the PE via gaussian
    symmetry gh=[a,b,c,b,a]: blur = (c*Gv)@x0 + (b*Gv)@s1 + (a*Gv)@s2 with
    s1=x[-1]+x[+1], s2=x[-2]+x[+2]. fp32 is mandatory: f32r/bf16/f16 matmuls
    measure ~11-bit mantissa on HW, flipping ~10k near-tie NMS compares (the
    2e-2 rel-err budget only allows ~500).
  - s1/s2 pre-adds on the Pool engine (gpsimd; HW ISA = add/mult/memset only,
    max/is_ge/stt/copy are rejected), ACT does PSUM->SBUF blur copies +
    reflect edge-column scale-copies.
  - vertical 3x1 max on DVE: two DMA-materialized partition-shifted copies
    (engines cannot read operands at mismatched partition offsets), then
    t1 = max(blur, blurdn, THRP) (stt) and vm = max(t1, blur2).
  - horizontal max + NMS compare + select: the 3 GATE instructions above.
Schedule: 3-stage software pipeline (emission order = tile-scheduler
priority): consume_mm(k) emitted BEFORE produce(k+LEAD) each step so the
shift-copy DMAs of iter k enqueue ahead of the input load of iter k+LEAD on
the serialized DMA device (the chain's critical dependency); produce leads
consume_mm by LEAD=2 (xin/ssum bufs=LEAD+2 breaks the Pool<->PE circular
wait through the s1/s2 buffer rotation that periodically stalled PE and
reset its p-state ramp; xin at LEAD+3=5 bufs gives the input loads extra
slack on the serialized DMA device, -701ns), chain trails by PRE=1. Shift copies issue from the
ACT queue (HWDGE), loads/stores from SP. DMA batched via transposed DRAM
access patterns. The first two iterations' s1/s2 run on the DVE (idle during
pipeline fill, and 2x Pool's rate), pulling the j=1/j=0 matmuls earlier;
iters 0-1 load x in 1-PLANE DMA slices (range-level tile deps let each
plane's matmul start as soon as its slice lands, cascading the fill ~4.5us
earlier) and iters 0-3 run the DVE chain 2-plane-sliced (starts on planes
0-1 before 2-3's shifts land);
the last iteration's chain+store is 2-plane-sliced so the final store
overlaps the final gates (drain); ALL stores issue as 2-plane slices for
finer interleave on the serialized DMA device (-200ns; 1-plane is worse).
TimelineSim: 300597ns e2e (DVE 275us busy at 97.5% occupancy = the wall:
t1/vm/3 gates x 32 iters; PE 262.8 near its 254.6 fp32 floor - 4 cyc/row,
p-state-ramped, and any hi/lo split scheme at equal precision also needs
>=4 cyc/row so fp32 is optimal; DMA-device 225 serialized at 360GB/s
aggregate; Pool ~215; ACT 92) vs 339985ns for the previous t2/m2/is_ge/mult
kernel; verified rel err 6.032e-3 (bit-identical mismatch stats to v1).
Known floors: DVE 2-tensor ops are 1 cyc/elem fp32 (2x_2p half-cycle mode
applies only to 1-tensor ops: TensorCopy/TensorScalar); the 9-cell compare
needs >=5 two-stream passes (coverage doubling argument); remaining e2e gap
is ~27.6us pipeline fill, mostly PE p-state ramp physics + load latency
(slicing iter 0 finer was tried and made the sim schedule worse).
"""

import numpy as np

B, C, H, W = 256, 2, 224, 398
N_CORES = 8
PLANES = B * C                    # 512
P_CORE = PLANES // N_CORES        # 64 planes per core
GRP = 4                           # planes per supertile
KS = 5
SIGMA = 2.0
THR = 0.3

# chunk geometry: (raw_row0, out_row0)
#  chunk 0: blur rows 0..112   (ext: [b0, b0..b112]),  raw rows 0..114
#  chunk 1: blur rows 111..223 (ext: [b111..b223, b223]), raw rows 109..223
CHUNKS = [(0, 0), (109, 112)]
KDIM = 115                        # raw input rows per chunk
MDIM = 114                        # ext blur rows per chunk (113 + 1 dup)
OUTR = 112                        # output rows per chunk
NEGPAD = -1.0e30                  # vm pad columns (acts as maxpool -inf pad)

_nc_cache = {}


def _gauss():
    x = np.arange(KS, dtype=np.float32) - np.float32((KS - 1) / 2.0)
    g = np.exp(np.float32(-0.5) * (x / np.float32(SIGMA)) ** 2).astype(np.float32)
    g = (g / g.sum()).astype(np.float32)
    return g


def _gmats():
    """lhsT matrices [2 chunks, 5 shifts, K=115, M=114] fp32, then packed
    to [115, 2*5*114] (partition dim = K first)."""
    g = _gauss()

    def refl(r):
        if r < 0:
            return -r
        if r >= H:
            return 2 * H - 2 - r
        return r

    out = np.zeros((2, KS, KDIM, MDIM), np.float32)
    for c, (raw0, _) in enumerate(CHUNKS):
        for m in range(MDIM):
            if c == 0:
                br = max(m - 1, 0)            # ext[0] duplicates blur row 0
            else:
                br = 111 + min(m, MDIM - 2)   # ext[113] duplicates blur row 223
            for i in range(KS):
                k = refl(br + i - 2) - raw0
                assert 0 <= k < KDIM
                for j in range(KS):
                    out[c, j, k, m] += g[i] * g[j]
    return np.ascontiguousarray(out.transpose(2, 0, 1, 3).reshape(KDIM, 2 * KS * MDIM))


def _gate_op():
    """Register (once) and return the custom DVE op GATE(a,b) = b * (b >= a).

    Follows the documented extension path for concourse.dve_ops (define a
    DveOp, give it an opcode row, list it in OPS/CUSTOM_DVE_SPECS) but does
    it at runtime since the repo tree is read-only. The uops sha is pinned
    from this process's own lower() output (same call compile() makes).
    """
    from concourse import dve_ops as dvo
    from concourse.dve_spec import Spec, Src0, Src1, lower
    from concourse.dve_uop import DveOpSpec

    name = "NMS_GATE_ANT"
    if name in dvo._SUB_OPCODE_FOR_NAME:
        return next(op for op in dvo.OPS if op.name == name)

    spec = Spec(
        body=Src1 * (Src1 >= Src0),
        reference=lambda in0, in1, s0, s1, imm2: (
            in1.astype(np.float32) * (in1 >= in0)
        ).astype(np.float32),
    )
    row = max(dvo._SUB_OPCODE_FOR_NAME.values()) + 1
    assert row < 0x20, "no free custom-DVE opcode rows"
    dvo._SUB_OPCODE_FOR_NAME[name] = row
    shas = {}
    for ver in ("v3", "v4"):
        uops = lower(spec, ver=ver)
        shas[ver] = DveOpSpec(name=name, opcode=row, uops=uops, rd1_en=True).sha(ver)
    op = dvo.DveOp(name, spec, subdim=False, uops_sha=shas)
    dvo.OPS.append(op)
    dvo.CUSTOM_DVE_SPECS[name] = spec
    return op


def _build():
    import concourse.bacc as bacc
    import concourse.tile as tile
    import concourse.mybir as mybir

    f32 = mybir.dt.float32
    bf16 = mybir.dt.bfloat16
    AOT = mybir.AluOpType
    ACT = mybir.ActivationFunctionType
    THRP = float(np.nextafter(np.float32(THR), np.float32(1.0)))
    GATE = _gate_op()

    nc = bacc.Bacc(trn_type="TRN2", target_bir_lowering=False, debug=False)
    x_t = nc.dram_tensor("x", [P_CORE, H, W], f32, kind="ExternalInput")
    g_t = nc.dram_tensor("g", [KDIM, 2 * KS * MDIM], f32, kind="ExternalInput")
    o_t = nc.dram_tensor("o", [P_CORE, H, W], bf16, kind="ExternalOutput")
    x_ap = x_t.ap()
    o_ap = o_t.ap()

    NGRP = P_CORE // GRP
    IT = [(grp, ci) for grp in range(NGRP) for ci in range(2)]
    LEAD = 2   # produce runs LEAD steps ahead of consume_mm (keeps PE fed)
    PRE = 1    # consume_chain trails consume_mm by PRE steps

    with tile.TileContext(nc) as tc:
        with tc.tile_pool(name="const", bufs=1) as constp, \
             tc.tile_pool(name="xin", bufs=LEAD + 3) as xinp, \
             tc.tile_pool(name="ssum", bufs=LEAD + 2) as ssump, \
             tc.tile_pool(name="work", bufs=3) as workp, \
             tc.tile_pool(name="ps", bufs=2, space="PSUM") as psp:
            gt = constp.tile([KDIM, 2 * KS * MDIM], f32, tag="g")
            nc.gpsimd.dma_start(out=gt[:], in_=g_t.ap())
            state = {}
            mmstate = {}

            def produce(it):
                grp, c = it
                raw0, _ = CHUNKS[c]
                p0 = grp * GRP
                # ---- load input tile (one batched DMA per group) ----
                xt = xinp.tile([KDIM, GRP, W], f32, tag="x")
                ld_slices = (
                    [slice(i, i + 1) for i in range(GRP)]
                    if IT.index(it) < 2
                    else [slice(0, GRP)]
                )
                for ld in ld_slices:
                    nc.sync.dma_start(
                        out=xt[:, ld, :],
                        in_=x_ap[
                            p0 + ld.start : p0 + ld.stop, raw0 : raw0 + KDIM, :
                        ].transpose([1, 0, 2]),
                    )
                # ---- shifted-sum tiles (exact fp32): s1[c]=x[c-1]+x[c+1],
                # s2[c]=x[c-2]+x[c+2]; horizontal reflect folds into the edge
                # columns as 2*x[k] (ACT scale-copies) or interior pairs.
                seng = nc.vector if IT.index(it) < 2 else nc.gpsimd
                s1 = ssump.tile([KDIM, GRP, W], f32, tag="s1", name=f"s1_{grp}_{c}")
                s2 = ssump.tile([KDIM, GRP, W], f32, tag="s2", name=f"s2_{grp}_{c}")
                for ld in ld_slices:
                    seng.tensor_tensor(
                        s1[:, ld, 1 : W - 1], xt[:, ld, 0 : W - 2], xt[:, ld, 2:W],
                        AOT.add,
                    )
                    seng.tensor_tensor(
                        s2[:, ld, 2 : W - 2], xt[:, ld, 0 : W - 4], xt[:, ld, 4:W],
                        AOT.add,
                    )
                state[it] = (xt, s1, s2)

            def consume_mm(it):
                grp, c = it
                raw0, out0 = CHUNKS[c]
                p0 = grp * GRP
                xt, s1, s2 = state[it]
                first = grp == 0
                last = grp == NGRP - 1
                # reflect edge columns of s1/s2, emitted here (not in
                # produce) so no engine queue ever waits on a future load
                nc.scalar.activation(s1[:, :, 0:1], xt[:, :, 1:2], ACT.Copy, scale=2.0)
                nc.scalar.activation(
                    s1[:, :, W - 1 : W], xt[:, :, W - 2 : W - 1], ACT.Copy, scale=2.0
                )
                nc.scalar.activation(s2[:, :, 0:1], xt[:, :, 2:3], ACT.Copy, scale=2.0)
                nc.scalar.activation(
                    s2[:, :, W - 1 : W], xt[:, :, W - 3 : W - 2], ACT.Copy, scale=2.0
                )
                nc.gpsimd.tensor_tensor(
                    s2[:, :, 1:2], xt[:, :, 1:2], xt[:, :, 3:4], AOT.add
                )
                nc.gpsimd.tensor_tensor(
                    s2[:, :, W - 2 : W - 1], xt[:, :, W - 4 : W - 3],
                    xt[:, :, W - 2 : W - 1], AOT.add,
                )

                # ---- full separable blur on PE: 3 accumulating matmuls ----
                pss = [
                    psp.tile([MDIM, 512], f32, tag=f"p{i}", name=f"ps_{grp}_{c}_{i}")
                    for i in range(GRP)
                ]
                # j=2 (center, no s1/s2 dependency) first for overlap: the
                # center matmuls need only xt, covering the s1/s2 Pool latency.
                if it == IT[0]:
                    order = [(2, i) for i in range(GRP)] + [
                        (j, i) for i in range(GRP) for j in (1, 0)
                    ]
                else:
                    order = [(j, i) for j in (2, 1, 0) for i in range(GRP)]
                for j, i in order:
                    term = (2, 1, 0).index(j)
                    lhs = gt[:, (c * KS + j) * MDIM : (c * KS + j + 1) * MDIM]
                    if j == 2:
                        rhs = xt[:, i, :]
                    elif j == 1:
                        rhs = s1[:, i, :]
                    else:
                        rhs = s2[:, i, :]
                    nc.tensor.matmul(
                        out=pss[i][:, 0:W],
                        lhsT=lhs,
                        rhs=rhs,
                        start=(term == 0),
                        stop=(term == 2),
                    )

                # ---- PSUM -> SBUF (ACT), plus shifted copies via DMA ----
                blur = workp.tile([MDIM, GRP, 400], f32, tag="blur")
                for i in range(GRP):
                    nc.scalar.copy(blur[:, i, 0:W], pss[i][:, 0:W])
                pl_slices = [slice(0, 2), slice(2, 4)]
                # blurdn[r] = ext[r+1]: the center row values for out row r
                # (also the t1 operand); blur2[r] = ext[r+2].
                blurdn = workp.tile([OUTR, GRP, 400], f32, tag="blurdn")
                blur2 = workp.tile([OUTR, GRP, 400], f32, tag="blur2")
                for sl in pl_slices:
                    nc.scalar.dma_start(
                        out=blurdn[:, sl, 0:W], in_=blur[1 : OUTR + 1, sl, 0:W]
                    )
                    nc.scalar.dma_start(
                        out=blur2[:, sl, 0:W], in_=blur[2 : OUTR + 2, sl, 0:W]
                    )
                # vm pad columns (survive all iterations of this tile buffer;
                # cheap Pool memsets, re-done per iter since pool bufs rotate)
                vm = workp.tile([OUTR, GRP, 400], f32, tag="vm")
                nc.gpsimd.memset(vm[:, :, 0:1], NEGPAD)
                nc.gpsimd.memset(vm[:, :, 399:400], NEGPAD)
                mmstate[it] = (blur, blurdn, blur2, vm)

            def consume_chain(it):
                grp, c = it
                raw0, out0 = CHUNKS[c]
                p0 = grp * GRP
                blur, blurdn, blur2, vm = mmstate.pop(it)
                state.pop(it)
                t1 = workp.tile([OUTR, GRP, 400], f32, tag="t1")
                outv = workp.tile([OUTR, GRP, W], bf16, tag="outv")
                # e1 reuses t1's buffer, e2 reuses blur2's (same-engine
                # in-order WAR: t1/blur2 are last read by the vm instruction,
                # e1/e2 written by later DVE instructions).
                e1 = t1
                e2 = blur2
                ch_slices = (
                    [slice(0, 2), slice(2, 4)]
                    if it in IT[:4] or it == IT[-1]
                    else [slice(0, GRP)]
                )
                st_slices = [slice(0, 2), slice(2, 4)]
                for sl in ch_slices:
                    # ---- vertical 3x1 max (+ threshold fold) on DVE ----
                    nc.vector.scalar_tensor_tensor(
                        out=t1[:, sl, 0:W],
                        in0=blur[0:OUTR, sl, 0:W],
                        scalar=THRP,
                        in1=blurdn[:, sl, 0:W],
                        op0=AOT.max,
                        op1=AOT.max,
                    )
                    # vm data lands in cols 1..398 (pads at 0 and 399)
                    nc.vector.tensor_tensor(
                        vm[:, sl, 1 : W + 1], t1[:, sl, 0:W], blur2[:, sl, 0:W],
                        AOT.max,
                    )
                    # ---- horizontal max + NMS compare + select: 3 GATEs ----
                    nc.vector._custom_dve(
                        GATE, out=e1[:, sl, 0:W],
                        in0=vm[:, sl, 1 : W + 1], in1=blurdn[:, sl, 0:W],
                    )
                    nc.vector._custom_dve(
                        GATE, out=e2[:, sl, 0:W],
                        in0=vm[:, sl, 2 : W + 2], in1=e1[:, sl, 0:W],
                    )
                    nc.vector._custom_dve(
                        GATE, out=outv[:, sl, 0:W],
                        in0=vm[:, sl, 0:W], in1=e2[:, sl, 0:W],
                    )
                for st in (st_slices if len(ch_slices) == 1 else ch_slices):
                    nc.sync.dma_start(
                        out=o_ap[
                            p0 + st.start : p0 + st.stop, out0 : out0 + OUTR, :
                        ].transpose([1, 0, 2]),
                        in_=outv[:, st, 0:W],
                    )

            for step in range(len(IT) + LEAD + PRE):
                if LEAD <= step < len(IT) + LEAD:
                    consume_mm(IT[step - LEAD])
                if step < len(IT):
                    produce(IT[step])
                if step >= LEAD + PRE:
                    consume_chain(IT[step - LEAD - PRE])

    nc.compile()
    return nc


def _make_sharded():
    """Build the shard_map'd PJRT executable ONCE and cache it, so repeat
    kernel() calls skip jit re-tracing / recompilation (~6s/call)."""
    import jax
    from jax.sharding import Mesh, PartitionSpec, NamedSharding
    from jax.experimental.shard_map import shard_map
    import concourse.mybir as mybir
    from concourse import bass2jax
    from concourse.bass2jax import _bass_exec_p, install_neuronx_cc_hook

    nc = _nc_cache["nc"]
    install_neuronx_cc_hook()
    partition_name = nc.partition_id_tensor.name if nc.partition_id_tensor else None
    in_names, out_names, out_avals, zero_shapes = [], [], [], []
    for alloc in nc.m.functions[0].allocations:
        if not isinstance(alloc, mybir.MemoryLocationSet):
            continue
        name = alloc.memorylocations[0].name
        if alloc.kind == "ExternalInput":
            if name != partition_name:
                in_names.append(name)
        elif alloc.kind == "ExternalOutput":
            out_names.append(name)
            shape = tuple(alloc.tensor_shape)
            dtype = mybir.dt.np(alloc.dtype)
            out_avals.append(jax.core.ShapedArray(shape, dtype))
            zero_shapes.append((shape, dtype))
    n_params = len(in_names)
    n_outs = len(out_avals)
    all_in_names = list(in_names) + list(out_names)
    if partition_name is not None:
        all_in_names.append(partition_name)

    def _body(*args):
        operands = list(args)
        if partition_name is not None:
            operands.append(bass2jax.partition_id_tensor())
        return tuple(_bass_exec_p.bind(
            *operands,
            out_avals=tuple(out_avals),
            in_names=tuple(all_in_names),
            out_names=tuple(out_names),
            lowering_input_output_aliases=(),
            sim_require_finite=True,
            sim_require_nnan=True,
            nc=nc,
        ))

    devices = jax.devices()[:N_CORES]
    mesh = Mesh(np.asarray(devices), ("core",))
    sharded = jax.jit(
        shard_map(
            _body, mesh=mesh,
            in_specs=(PartitionSpec("core"),) * (n_params + n_outs),
            out_specs=(PartitionSpec("core"),) * len(out_names),
            check_rep=False,
        ),
        donate_argnums=tuple(range(n_params, n_params + n_outs)),
        keep_unused=True,
    )
    sh = NamedSharding(mesh, PartitionSpec("core"))
    return sharded, sh, in_names, out_names, zero_shapes


def kernel(hands_batch: np.ndarray) -> np.ndarray:
    import jax

    x = np.ascontiguousarray(np.asarray(hands_batch, dtype=np.float32))
    assert x.shape == (B, C, H, W)

    if "nc" not in _nc_cache:
        _nc_cache["nc"] = _build()
        _nc_cache["g"] = _gmats()
        _nc_cache["fn"] = _make_sharded()
    sharded, sh, in_names, out_names, zero_shapes = _nc_cache["fn"]
    gm = _nc_cache["g"]

    concat = {
        "x": x.reshape(N_CORES * P_CORE, H, W),
        "g": np.concatenate([gm] * N_CORES, axis=0),
    }
    args = [jax.device_put(concat[nm], sh) for nm in in_names]
    zeros = [
        jax.device_put(np.zeros((N_CORES * s[0], *s[1:]), d), sh)
        for (s, d) in zero_shapes
    ]
    outs = sharded(*args, *zeros)
    out = np.asarray(outs[out_names.index("o")]).astype(np.float32)
    return out.reshape(B, C, H, W)


if __name__ == "__main__":
    rng = np.random.default_rng(0)
    x = rng.random((B, C, H, W), dtype=np.float32)
    y = kernel(x)
    print("kernel ran, out shape", y.shape, "nonzero frac", (y != 0).mean())
